# revision 49
# baseline (speedup 1.0000x reference)
"""Trainium2 Bass kernel for ConstrastiveCrossViewLucasVSCorineLoss.

Math (see the reference):
  corine = label[:, ::4, ::4].flatten()                       # [N], N=65536
  feats  = features.transpose(0,2,3,1).reshape(N, 768)
  sums/counts = per-class segment sums of feats over corine   # [9,768], [9]
  protos = l2norm(0.99*sums/counts + 0.01*prototypes)         # [9,768]
  logits = protos @ feats.T                                   # [9,N]
  pf     = l2norm(logits, axis=-1) / 0.1 ; pf[2] = (corine7to6 == 2)
  loss   = mean(log(sum_c exp(pf[c,i])) - pf[l_i, i])

Because pf row-normalizes the logits, any per-row scaling of the protos
cancels: the l2norm of the protos is dropped and the raw
pp = 0.99*sums/counts + 0.01*proto0 is used directly.

Sharding: data-parallel over N across 8 cores (each core: half of one
batch, 8192 columns).  Host pre-casts the feature shard to bf16 (same
rounding the device would do), so each core streams only 12.6 MB and the
chunk lands straight in the resident [128, 6, cols] SBUF tile - no
staging, no downcast pass.  Per 128-column block, PE transposes the six
d-tiles into one PSUM bank and accumulates class sums with the one-hot
label matrix as the stationary operand (seg-matmuls pipelined one block
behind the transposes to keep PE gapless).  The class sums go through a
single bf16 AllGather; the gathered [72,768] is reduced on PE, which
also produces the remapped G sums for the A1 term:
  sum_i pf[l_i, i] = sum_c s_c * <pp[c], G[c]>,  G[6]=S[6]+S[7], G[7]=0.
Phase B issues all logits matmuls back-to-back (lets the PE p-state ramp
to 2.4 GHz), with the per-row sum-of-squares on gpsimd and the bf16 copy
on DVE trailing behind; the row-sumsq AllGather is triggered before the
logit transposes so it overlaps them.  The tail runs in the transposed
[1,9]/[128,nblk,9] layout throughout: rsqrt+mask of the gathered row
norms, PE broadcast, mult+exp+logsumexp, two tiny dots, one scalar out.
Host sums the 8 partials and subtracts the class-2 count.
"""

import sys
import types

import ml_dtypes
import numpy as np

# The image's antenv lacks axon_hooks; run_bass_kernel_spmd imports it when
# tracing.  Provide an inert shim so the import never breaks (trace off here).
if "antenv.axon_hooks" not in sys.modules:
    _m = types.ModuleType("antenv.axon_hooks")
    _m._hook = None
    _m.set_axon_ntff_profile_hook = lambda h: setattr(_m, "_hook", h)
    _m.get_axon_ntff_profile_hook = lambda: _m._hook
    sys.modules["antenv.axon_hooks"] = _m

import concourse.bacc as bacc
import concourse.mybir as mybir
import concourse.tile as tile
from concourse import bass_utils
from concourse.bass import broadcast_tensor_aps
from concourse.masks import make_identity

F32 = mybir.dt.float32
BF16 = mybir.dt.bfloat16
FP8 = mybir.dt.float8e4
ALU = mybir.AluOpType
ACTF = mybir.ActivationFunctionType
DROW = mybir.MatmulPerfMode.DoubleRow

# Phase-B logits in fp8 with DoubleRow (2 MACs/cycle).  The prototypes are
# scaled by PSC so their ~1e-2 entries land in e4m3's normal range; the scale
# cancels in the row normalization and is compensated in mrT for the A1 term.
FP8B = True
PSC = 64.0

N_CORES = 8
B, D, H, W = 4, 768, 128, 128
NUM_CLASSES = 9
N_TOTAL = B * H * W          # 65536
COLS = N_TOTAL // N_CORES    # 8192 columns per core
CH = 1024                    # columns per input DMA chunk
ALPHA = 0.99
TEMP = 0.1
NTILE = D // 128             # 6
E_M1 = float(np.e - 1.0)

STAGES = ("A", "C1", "P", "B", "full")


def build(cols=COLS, ch=CH, stage="full", fp8=FP8B):
    assert cols % 512 == 0 and cols % ch == 0 and ch % 128 == 0
    assert stage in STAGES
    nch = cols // 128          # 128-col blocks
    njc = cols // ch           # DMA chunks
    n512 = cols // 512         # phase-B groups
    nblk = nch                 # transposed-logit blocks

    nc = bacc.Bacc("TRN2", target_bir_lowering=False, debug=False, num_devices=N_CORES)
    feat = nc.dram_tensor("feat", [D, cols], BF16, kind="ExternalInput").ap()
    onehot_l = nc.dram_tensor("onehot_l", [128, nch, 9], BF16, kind="ExternalInput").ap()
    ind01_in = nc.dram_tensor("ind01", [128, nblk], BF16, kind="ExternalInput").ap()
    rc99_in = nc.dram_tensor("rc99", [9, 1], F32, kind="ExternalInput").ap()
    q01_in = nc.dram_tensor("q01", [9, D], F32, kind="ExternalInput").ap()
    bs72_in = nc.dram_tensor("bs72", [72, 9], BF16, kind="ExternalInput").ap()
    bsg72_in = nc.dram_tensor("bsg72", [72, 9], BF16, kind="ExternalInput").ap()
    mask01_in = nc.dram_tensor("mask01", [1, 9], F32, kind="ExternalInput").ap()
    out = nc.dram_tensor("out", [1, 1], F32, kind="ExternalOutput").ap()

    cc1_in = nc.dram_tensor("cc1_in", [9, D], BF16).ap()
    cc1_out = nc.dram_tensor("cc1_out", [72, D], BF16, addr_space="Shared").ap()
    cc2_in = nc.dram_tensor("cc2_in", [1, 128], F32).ap()
    cc2_out = nc.dram_tensor("cc2_out", [8, 128], F32, addr_space="Shared").ap()

    groups = [list(range(N_CORES))]
    feat_v = feat.rearrange("(t p) n -> p t n", p=128)

    with tile.TileContext(nc) as tc:
        with (
            tc.tile_pool(name="singles", bufs=1) as singles,
        ):
            res = singles.tile([128, NTILE, cols], BF16, tag="res")
            res8 = None
            if fp8:
                res8 = singles.tile(
                    [128, NTILE // 2, 2, cols], FP8, name="res8", tag="res8"
                )

            # ---- chunk schedule: small first chunk (engage compute early),
            # big 4KB-line chunks in the middle, tapered tail for the drain
            if cols == 8192:
                chunk_lens = [256, 512, 1024, 2048, 2048, 1024, 512, 512, 256]
            else:
                chunk_lens = [ch] * njc
                if njc >= 2 and ch % 256 == 0:
                    chunk_lens = [ch] * (njc - 1) + [ch // 4] * 4
            chunk_offs = list(np.cumsum([0] + chunk_lens[:-1]))
            # kick the first chunk before anything else queues on sync
            nc.sync.dma_start(
                out=res[:, :, chunk_offs[0] : chunk_offs[0] + chunk_lens[0]],
                in_=feat_v[:, :, chunk_offs[0] : chunk_offs[0] + chunk_lens[0]],
            )

            # ---- identity + small constants (gpsimd queue, overlap chunk 0)
            ident = singles.tile([128, 128], F32, tag="ident")
            make_identity(nc, ident)
            identb = singles.tile([128, 128], BF16, tag="identb")
            nc.vector.tensor_copy(identb, ident)
            ones1r = singles.tile([1, 128], F32, tag="ones1r")
            nc.vector.memset(ones1r, 1.0)
            onesc = singles.tile([128, 1], F32, tag="onesc")
            nc.vector.memset(onesc, 1.0)
            ones8 = singles.tile([8, 1], F32, tag="ones8")
            nc.vector.memset(ones8, 1.0)
            dumi = singles.tile([1, 1], F32, tag="dumi")
            nc.vector.memset(dumi, 1.0)
            dumo = singles.tile([1, 1], F32, tag="dumo")
            oh = singles.tile([128, nch, 9], BF16, tag="oh")
            nc.gpsimd.dma_start(out=oh, in_=onehot_l)
            ind01 = singles.tile([128, nblk], BF16, tag="ind01")
            nc.gpsimd.dma_start(out=ind01, in_=ind01_in)
            rc99 = singles.tile([9, 1], F32, tag="rc99")
            nc.gpsimd.dma_start(out=rc99, in_=rc99_in)
            q01 = singles.tile([9, D], F32, tag="q01")
            nc.gpsimd.dma_start(out=q01, in_=q01_in)
            bs72 = singles.tile([72, 9], BF16, tag="bs72")
            nc.gpsimd.dma_start(out=bs72, in_=bs72_in)
            bsg72 = singles.tile([72, 9], BF16, tag="bsg72")
            nc.gpsimd.dma_start(out=bsg72, in_=bsg72_in)
            mask01 = singles.tile([1, 9], F32, tag="mask01")
            nc.gpsimd.dma_start(out=mask01, in_=mask01_in)

            sums_bf = singles.tile([9, D], BF16, tag="sums_bf")

            # ---- Phase A: stream feats, PE-transpose, one-hot segment sums.
            # Seg-matmuls run one block behind the transposes so PE never
            # stalls on the DVE psum->sbuf copy.
            with (
                tc.tile_pool(name="psums", bufs=1, space="PSUM") as psums_pool,
                tc.tile_pool(name="psT", bufs=4, space="PSUM") as psT_pool,
                tc.tile_pool(name="trans", bufs=6) as trans_pool,
            ):
                ps_sums = psums_pool.tile([9, D], F32, tag="ps_sums")
                pending = None

                def seg_mm(p, half):
                    gnb, tr = p
                    first, last = gnb == 0, gnb == nch - 1
                    lhs = oh[:, gnb, :]
                    if half == 0:
                        nc.tensor.matmul(ps_sums[:, 0:512], lhsT=lhs, rhs=tr[:, 0:4, :],
                                         start=first, stop=last)
                    else:
                        nc.tensor.matmul(ps_sums[:, 512:768], lhsT=lhs, rhs=tr[:, 4:6, :],
                                         start=first, stop=last)

                for j, (joff, jlen) in enumerate(zip(chunk_offs, chunk_lens)):
                    if j > 0:
                        nc.sync.dma_start(
                            out=res[:, :, joff : joff + jlen],
                            in_=feat_v[:, :, joff : joff + jlen],
                        )
                    if fp8:
                        # fp8 shadow of the resident for the DoubleRow logits
                        # (scalar engine is otherwise idle in phase A)
                        for t in range(NTILE):
                            nc.scalar.copy(
                                res8[:, t // 2, t % 2, joff : joff + jlen],
                                res[:, t, joff : joff + jlen],
                            )
                    for nb in range(jlen // 128):
                        gnb = joff // 128 + nb
                        gsl = slice(gnb * 128, (gnb + 1) * 128)
                        psT = psT_pool.tile([128, NTILE, 128], BF16, tag="psT")
                        # interleave the previous block's seg-matmuls between
                        # transposes so their streams cover the weight loads
                        for t in range(NTILE):
                            nc.tensor.transpose(psT[:, t, :], res[:, t, gsl], identb)
                            if pending is not None and t in (1, 4):
                                seg_mm(pending, 0 if t == 1 else 1)
                        tr = trans_pool.tile([128, NTILE, 128], BF16, tag="tr")
                        nc.vector.tensor_copy(tr, psT)
                        pending = (gnb, tr)
                seg_mm(pending, 0)
                seg_mm(pending, 1)

                # ---- collective 1: bf16 AllGather of local class sums
                # (stage via sync - the stream is done, its queue is free)
                nc.vector.tensor_copy(sums_bf, ps_sums)
                nc.sync.dma_start(out=cc1_in, in_=sums_bf)
                nc.gpsimd.collective_compute(
                    "AllGather", ALU.bypass, replica_groups=groups,
                    ins=[cc1_in], outs=[cc1_out],
                )
                # preload activation tables during the collective wait (the
                # tail uses Sqrt then Exp; Ln is a DVE polynomial)
                nc.scalar.activation(dumo, dumi, ACTF.Exp)
                nc.scalar.activation(dumo, dumi, ACTF.Sqrt)

            stage_done = stage == "A"
            if stage == "A":
                nc.sync.dma_start(out=out, in_=rc99[0:1, 0:1])

            if not stage_done:
                # fetch the gathered sums in pair-sized slices so the P-phase
                # pipeline starts on the first 256 columns ASAP
                gath = singles.tile([72, D], BF16, tag="gath")
                for pr in range(NTILE // 2):
                    dsl = slice(pr * 256, (pr + 1) * 256)
                    nc.sync.dma_start(out=gath[:, dsl], in_=cc1_out[:, dsl])

            if stage == "C1":
                nc.sync.dma_start(out=out, in_=rc99[0:1, 0:1])
                stage_done = True

            if fp8:
                protosT8 = singles.tile([128, NTILE // 2, 2, 16], FP8, tag="protosT8")
            else:
                protosT = singles.tile([128, NTILE, 9], BF16, tag="protosT")
            rowdot9 = singles.tile([9, 1], F32, tag="rowdot9")
            mrT = singles.tile([1, 9], F32, tag="mrT")
            if not stage_done:
                # ---- P phase: global sums, pp = rc99*S + q01 (no l2norm - it
                # cancels in the row normalization), transpose for phase B,
                # G-trick row dots for the A1 term.
                pp = singles.tile([9, D], BF16, tag="pp")
                junkG = singles.tile([9, D], BF16, tag="junkG")
                with (
                    tc.tile_pool(name="psP", bufs=1, space="PSUM") as psP_pool,
                    tc.tile_pool(name="psTp", bufs=2, space="PSUM") as psTp_pool,
                ):
                    # pipeline the global-sum reduce / pp / transpose in
                    # 256-wide pairs so phase B's first matmul starts early
                    psS = psP_pool.tile([9, D], F32, tag="psS")
                    for pr in range(NTILE // 2):
                        dsl = slice(pr * 256, (pr + 1) * 256)
                        nc.tensor.matmul(psS[:, dsl], lhsT=bs72, rhs=gath[:, dsl],
                                         start=True, stop=True)
                        nc.vector.scalar_tensor_tensor(
                            out=pp[:, dsl], in0=psS[:, dsl], scalar=rc99,
                            in1=q01[:, dsl], op0=ALU.mult, op1=ALU.add,
                        )
                        for t in (2 * pr, 2 * pr + 1):
                            psTp = psTp_pool.tile([128, 9], BF16, tag="psTp")
                            nc.tensor.transpose(
                                psTp, pp[:, t * 128 : (t + 1) * 128], identb[0:9, 0:9]
                            )
                            if fp8:
                                nc.vector.tensor_scalar_mul(
                                    protosT8[:, t // 2, t % 2, 0:9], psTp, PSC
                                )
                            else:
                                nc.vector.tensor_copy(protosT[:, t, :], psTp)
                    psG = psP_pool.tile([9, D], F32, tag="psG")
                    nc.tensor.matmul(psG[:, 0:512], lhsT=bsg72, rhs=gath[:, 0:512],
                                     start=True, stop=True)
                    nc.tensor.matmul(psG[:, 512:768], lhsT=bsg72, rhs=gath[:, 512:768],
                                     start=True, stop=True)
                    nc.vector.scalar_tensor_tensor(
                        out=junkG, in0=pp, scalar=1.0, in1=psG,
                        op0=ALU.mult, op1=ALU.mult, accum_out=rowdot9,
                    )
                    # mrT = rowdot^T / 8 (per-core share of the global A1
                    # term); fp8 logits are PSC x larger, so s9 is PSC x
                    # smaller and mrT compensates
                    psmr = psTp_pool.tile([1, 9], F32, tag="psmr")
                    nc.tensor.transpose(psmr, rowdot9, ident[0:9, 0:9])
                    nc.vector.tensor_scalar_mul(mrT, psmr, (PSC if fp8 else 1.0) / 8.0)

            if stage == "P":
                nc.sync.dma_start(out=out, in_=rowdot9[0:1, 0:1])
                stage_done = True

            lT = singles.tile([128, nblk, 9], BF16, tag="lT")
            sq = singles.tile([9, n512], F32, tag="sq")
            if not stage_done:
                # ---- Phase B: all logits matmuls back-to-back (PE p-state
                # ramp), sumsq on gpsimd + bf16 copy on DVE trailing behind,
                # then the sumsq AllGather, then the logit transposes (which
                # overlap the collective).
                lbf = singles.tile([9, n512, 512], BF16, tag="lbf")
                ssqT = singles.tile([1, 128], F32, tag="ssqT")
                nc.vector.memset(ssqT, 0.0)
                with (
                    tc.tile_pool(name="psL", bufs=5, space="PSUM") as psL_pool,
                    tc.tile_pool(name="psLT", bufs=2, space="PSUM") as psLT_pool,
                    tc.tile_pool(name="psq", bufs=1, space="PSUM") as psq_pool,
                    tc.tile_pool(name="jnk", bufs=2) as jnk_pool,
                ):
                    for g in range(n512):
                        psL = psL_pool.tile([9, 512], F32, tag="psL")
                        if fp8:
                            for t3 in range(NTILE // 2):
                                nc.tensor.matmul(
                                    psL, lhsT=protosT8[:, t3, :, 0:9],
                                    rhs=res8[:, t3, :, g * 512 : (g + 1) * 512],
                                    start=(t3 == 0), stop=(t3 == NTILE // 2 - 1),
                                    perf_mode=DROW,
                                )
                        else:
                            for t in range(NTILE):
                                nc.tensor.matmul(
                                    psL, lhsT=protosT[:, t, :],
                                    rhs=res[:, t, g * 512 : (g + 1) * 512],
                                    start=(t == 0), stop=(t == NTILE - 1),
                                )
                        nc.scalar.copy(lbf[:, g, :], psL)
                        # sumsq from the bf16 copy: frees the PSUM bank as
                        # soon as the scalar copy lands
                        junkB = jnk_pool.tile([9, 512], BF16, tag="junkB")
                        nc.vector.scalar_tensor_tensor(
                            out=junkB, in0=psL, scalar=1.0, in1=lbf[:, g, :],
                            op0=ALU.mult, op1=ALU.mult, accum_out=sq[:, g : g + 1],
                        )

                    # re-preload the Sqrt table (the lbf copies evicted it)
                    nc.scalar.activation(dumo, dumi, ACTF.Sqrt)

                    # ---- collective 2: AllGather the per-core row sumsq
                    # (padded to 512 B - tiny payloads hit a slow path)
                    ssq = singles.tile([9, 1], F32, tag="ssq")
                    nc.vector.tensor_reduce(
                        out=ssq, in_=sq, axis=mybir.AxisListType.X, op=ALU.add,
                    )
                    psq1 = psq_pool.tile([1, 9], F32, tag="psq1")
                    nc.tensor.transpose(psq1, ssq, ident[0:9, 0:9])
                    nc.vector.tensor_copy(ssqT[:, 0:9], psq1)
                    nc.sync.dma_start(out=cc2_in, in_=ssqT)
                    nc.gpsimd.collective_compute(
                        "AllGather", ALU.bypass, replica_groups=groups,
                        ins=[cc2_in], outs=[cc2_out],
                    )

                    # transpose logits into [128, nblk, 9] under the collective
                    for g in range(n512):
                        psLT = psLT_pool.tile([128, 4, 10], BF16, tag="psLT")
                        for i in range(4):
                            nc.tensor.transpose(
                                psLT[:, i, 0:9],
                                lbf[:, g, i * 128 : (i + 1) * 128],
                                identb[0:9, 0:9],
                            )
                        nc.vector.tensor_copy(lT[:, g * 4 : (g + 1) * 4, :], psLT[:, :, 0:9])

            if stage == "B":
                nc.sync.dma_start(out=out, in_=sq[0:1, 0:1])
                stage_done = True

            if not stage_done:
                # ---- tail: s = mask/(TEMP*||row||); pf; exp; logsumexp; out
                g2 = singles.tile([8, 9], F32, tag="g2")
                nc.sync.dma_start(out=g2, in_=cc2_out[:, 0:9])
                sroot = singles.tile([1, 9], F32, tag="sroot")
                srec = singles.tile([1, 9], F32, tag="srec")
                sTm = singles.tile([1, 9], F32, tag="sTm")
                pf_t = singles.tile([128, nblk, 9], BF16, tag="pf_t")
                ebf_t = singles.tile([128, nblk, 9], BF16, tag="ebf_t")
                a2 = singles.tile([128, nblk], F32, tag="a2")
                a2c = singles.tile([128, nblk], F32, tag="a2c")
                junk64 = singles.tile([128, nblk], BF16, tag="junk64")
                junk9 = singles.tile([1, 9], F32, tag="junk9")
                la2p = singles.tile([128, 1], F32, tag="la2p")
                r1 = singles.tile([1, 1], F32, tag="r1")
                df = singles.tile([1, 1], F32, tag="df")
                with tc.tile_pool(name="psE", bufs=1, space="PSUM") as psE_pool:
                    psg = psE_pool.tile([1, 9], F32, tag="psg")
                    nc.tensor.matmul(psg, lhsT=ones8, rhs=g2, start=True, stop=True)
                    nc.scalar.activation(sroot, psg, ACTF.Sqrt, scale=TEMP * TEMP)
                    # dummy exp: pull the Exp table load off the critical path
                    # (overlaps the DVE recip/mask/broadcast below)
                    nc.scalar.activation(dumo, dumi, ACTF.Exp)
                    nc.vector.reciprocal(srec, sroot)
                    nc.vector.tensor_mul(sTm, srec, mask01)
                    # r1 = sum_c s_c * rowdot_c / 8 (same value on every core)
                    nc.vector.scalar_tensor_tensor(
                        out=junk9, in0=sTm, scalar=1.0, in1=mrT,
                        op0=ALU.mult, op1=ALU.mult, accum_out=r1,
                    )
                    psbc = psE_pool.tile([128, 1, 9], F32, tag="psbc")
                    nc.tensor.matmul(psbc[:, 0, :], lhsT=ones1r, rhs=sTm,
                                     start=True, stop=True)
                    # mult/exp/reduce in two column halves so DVE and ACT pipeline
                    hb = nblk // 2
                    for h in range(2):
                        hs = slice(h * hb, (h + 1) * hb)
                        ap_lt, ap_sbc = broadcast_tensor_aps(
                            lT[:, hs, :], psbc[:, :, :]
                        )
                        nc.vector.tensor_tensor(
                            out=pf_t[:, hs, :], in0=ap_lt, in1=ap_sbc, op=ALU.mult
                        )
                        nc.scalar.activation(ebf_t[:, hs, :], pf_t[:, hs, :], ACTF.Exp)
                    for h in range(2):
                        hs = slice(h * hb, (h + 1) * hb)
                        nc.vector.tensor_reduce(
                            out=a2[:, hs], in_=ebf_t[:, hs, :],
                            axis=mybir.AxisListType.X, op=ALU.add,
                        )
                    # fix class 2: A2 += exp(ind) - exp(0) = (e-1)*ind
                    nc.vector.scalar_tensor_tensor(
                        out=a2c, in0=ind01, scalar=E_M1, in1=a2,
                        op0=ALU.mult, op1=ALU.add,
                    )
                    # sum_i ln(A2_i) = n*ln9 + sum_i log1p(x_i), x = A2/9 - 1,
                    # via a 4-term DVE Horner polynomial (|x| <= 0.25; the
                    # n*ln9 constant is added on the host).  Avoids the Ln
                    # activation-table load on the critical path.
                    xq = singles.tile([128, nblk], F32, tag="xq")
                    t1 = singles.tile([128, nblk], F32, tag="t1")
                    nc.vector.tensor_scalar(
                        out=xq, in0=a2c, scalar1=1.0 / 9.0, op0=ALU.mult,
                        scalar2=-1.0, op1=ALU.add,
                    )
                    nc.vector.tensor_scalar(
                        out=t1, in0=xq, scalar1=-0.25, op0=ALU.mult,
                        scalar2=1.0 / 3.0, op1=ALU.add,
                    )
                    nc.vector.scalar_tensor_tensor(
                        out=t1, in0=t1, scalar=1.0, in1=xq,
                        op0=ALU.mult, op1=ALU.mult,
                    )
                    nc.vector.scalar_tensor_tensor(
                        out=t1, in0=t1, scalar=-0.5, in1=xq,
                        op0=ALU.add, op1=ALU.mult,
                    )
                    nc.vector.scalar_tensor_tensor(
                        out=junk64, in0=t1, scalar=1.0, in1=xq,
                        op0=ALU.add, op1=ALU.mult, accum_out=la2p,
                    )
                    psr2 = psE_pool.tile([1, 1], F32, tag="psr2")
                    nc.tensor.matmul(psr2, lhsT=la2p, rhs=onesc, start=True, stop=True)
                    # df = sum_i log1p(x_i) - r1
                    nc.vector.scalar_tensor_tensor(
                        out=df, in0=r1, scalar=-1.0, in1=psr2,
                        op0=ALU.mult, op1=ALU.add,
                    )
                    nc.sync.dma_start(out=out, in_=df)
    nc.compile()
    return nc


def make_in_maps(features, corine, prototypes, cols=COLS):
    """Per-core input dicts. corine: [N] int labels; features: [B, D, n] f32."""
    n = corine.shape[0]
    n_cores = n // cols
    feats_flat = features.reshape(B, D, -1) if features.ndim == 4 else features
    lc = np.where(corine == 7, 6, corine)
    counts = np.bincount(corine, minlength=NUM_CLASSES).astype(np.float32)
    rc99 = (np.float32(ALPHA) / counts)[:, None]
    q01 = (np.float32(1.0) - np.float32(ALPHA)) * prototypes.astype(np.float32)
    mask01 = np.ones((1, NUM_CLASSES), np.float32)
    mask01[0, 2] = 0.0
    # bs72: reduce the AllGathered [72, D] partial sums to [9, D]
    p9 = np.arange(72) % 9
    bs72 = (p9[:, None] == np.arange(9)[None, :]).astype(ml_dtypes.bfloat16)
    # bsg72: same reduction composed with the 7->6 remap (row 6 <- S6+S7, row 7 <- 0)
    M = np.eye(NUM_CLASSES, dtype=np.float32)
    M[6, 7] = 1.0
    M[7, 7] = 0.0
    bsg72 = M.T[p9].astype(ml_dtypes.bfloat16)  # bsg72[p, c] = M[c, p%9]
    in_maps = []
    for c in range(n_cores):
        sl = slice(c * cols, (c + 1) * cols)
        lab = corine[sl]
        labc = lc[sl]
        oh_l = np.zeros((cols, NUM_CLASSES), np.float32)
        oh_l[np.arange(cols), lab] = 1.0
        oh_l = np.ascontiguousarray(
            oh_l.reshape(cols // 128, 128, NUM_CLASSES).transpose(1, 0, 2)
        ).astype(ml_dtypes.bfloat16)
        # ind01[p, b] = 1[labc[b*128+p] == 2] in the transposed-block layout
        e2 = (labc == 2).astype(np.float32)
        ind01 = np.ascontiguousarray(
            e2.reshape(cols // 128, 128).T
        ).astype(ml_dtypes.bfloat16)
        per_batch = feats_flat.shape[2]
        b, off = divmod(c * cols, per_batch)
        assert off + cols <= per_batch
        in_maps.append(
            {
                "feat": np.ascontiguousarray(
                    feats_flat[b][:, off : off + cols]
                ).astype(ml_dtypes.bfloat16),
                "onehot_l": oh_l,
                "ind01": ind01,
                "rc99": rc99,
                "q01": np.ascontiguousarray(q01),
                "bs72": bs72,
                "bsg72": bsg72,
                "mask01": mask01,
            }
        )
    return in_maps


def finalize(results, corine):
    """Combine per-core partials: add back the n*ln9 base of the device-side
    log1p polynomial, subtract the label-2 count A1 contribution."""
    n = corine.shape[0]
    lc = np.where(corine == 7, 6, corine)
    count2 = float((lc == 2).sum())
    total = sum(float(r["out"][0, 0]) for r in results) + n * np.log(9.0) - count2
    return total / n


_CACHED_NC = None


def kernel(cls_score, label, gt_lucas, features, prototypes):
    """Full-input entry point; cls_score and gt_lucas are unused by the math."""
    global _CACHED_NC
    label = np.asarray(label)
    features = np.asarray(features, dtype=np.float32)
    prototypes = np.asarray(prototypes, dtype=np.float32)
    corine = label[:, ::4, ::4].reshape(-1).astype(np.int32)
    if _CACHED_NC is None:
        _CACHED_NC = build()
    in_maps = make_in_maps(features, corine, prototypes)
    res = bass_utils.run_bass_kernel_spmd(
        _CACHED_NC, in_maps, core_ids=list(range(N_CORES))
    )
    return np.array(finalize(res.results, corine), dtype=np.float32)
